# revision 5
# baseline (speedup 1.0000x reference)
"""Trainium2 Bass kernel: dual-attention transformer block (nn_CustomBlock).

Reference semantics (per batch element b):
    q/k/v = x_b @ sa_w{q,k,v} + sa_b{q,k,v}
    sa    = softmax(q k^T / sqrt(DB)) v @ sa_wo + sa_bo
    x_b1  = x_b + sa
    q     = x_a @ ca_wq + ca_bq ; k/v = x_b1 @ ca_w{k,v} + ca_b{k,v}
    out   = x_b1 + softmax(q k^T / sqrt(DA)) v @ ca_wo + ca_bo

Sharding: data-parallel over batch — 8 batch elements, one per NeuronCore,
weights replicated.  No collectives.

Device kernel works in bf16 for all matmul operands (fp32 PSUM accumulation,
fp32 residual stream).  Exact host-side bias folding:
  - k-bias shifts every score row by a constant -> softmax-invariant -> dropped.
  - v-bias passes through attention unchanged (softmax weights sum to 1), so
    bv @ wo + bo folds into a single per-feature vector added to the residual
    input (SA) / the final output (CA) on the host.
  - q-bias is applied on device (per-partition bias in the q^T layout).

Softmax skips the max-subtraction: scores = q.k/sqrt(D) with these operand
scales stays in [-3, 3]; exp() in fp32 is safe by a wide margin.
"""

import math
from contextlib import ExitStack

import numpy as np
import ml_dtypes

import concourse.bass as bass
import concourse.mybir as mybir
import concourse.tile as tile
from concourse import bacc
from concourse.bass_utils import run_bass_kernel_spmd

P = 128
F32 = mybir.dt.float32
BF16 = mybir.dt.bfloat16
AF = mybir.ActivationFunctionType
ALU = mybir.AluOpType

B_FULL, N_FULL, DA_FULL, DB_FULL = 8, 2048, 768, 1024


def build_block(tc, outs, ins, n, da, db):
    """Emit the dual-attention block into TileContext `tc`.

    ins/outs: dicts of DRAM APs:
      ins:  xb_bf [n,db] bf16, xa_bf [n,da] bf16, xbpb [n,db] f32,
            sa_wq/sa_wk/sa_wv/sa_wo [db,db] bf16, ca_wq [da,db] bf16,
            ca_wk/ca_wv/ca_wo [db,db] bf16, bq_sa [P,db/P] f32, bq_ca [P,db/P] f32
      outs: out [n,db] f32
    """
    nc = tc.nc
    KB, KA, NI = db // P, da // P, n // P
    MC = min(512, n)          # projection m-chunk (columns of x^T)
    NMC = n // MC
    JH = min(1024, n)         # scores psum span (2 banks)
    NJH = n // JH
    JC = min(512, JH)         # one psum bank
    NJC = JH // JC
    SB = min(256, n)          # attention superblock (i columns per AV batch)
    NSB = n // SB
    IPSB = SB // P            # i-blocks per superblock
    EC = min(512, db)         # out-proj free chunk
    NEC = db // EC

    sc_sa = 1.0 / math.sqrt(float(db))
    sc_ca = 1.0 / math.sqrt(float(da))

    ctx = ExitStack()
    with ctx:
        sp = ctx.enter_context(tc.tile_pool(name="sp", bufs=1))
        pp = ctx.enter_context(tc.tile_pool(name="pp", bufs=1, space="PSUM"))
        dp = ctx.enter_context(tc.tile_pool(name="dp", bufs=1, space="DRAM"))

        # DRAM scratch
        qt_sa_d = dp.tile([db, n], BF16, tag="qt_sa")
        qt_ca_d = dp.tile([db, n], BF16, tag="qt_ca")
        xb1_d = dp.tile([n, db], F32, tag="xb1")
        xb1b_d = dp.tile([n, db], BF16, tag="xb1b")

        # persistent SBUF
        kT = sp.tile([P, KB, n], BF16, tag="kT")        # k^T  [feat, seq]
        v_sb = sp.tile([P, NI, db], BF16, tag="v")      # v    [seq, feat]
        bqs = sp.tile([P, KB], F32, tag="bqs")
        bqc = sp.tile([P, KB], F32, tag="bqc")
        zb = sp.tile([P, 1], F32, tag="zb")
        nc.gpsimd.dma_start(bqs[:], ins["bq_sa"][:])
        nc.gpsimd.dma_start(bqc[:], ins["bq_ca"][:])
        nc.gpsimd.memset(zb[:], 0.0)

        def load_w(name, ktiles):
            wt = sp.tile([P, ktiles, db], BF16, tag="w", bufs=2)
            nc.gpsimd.dma_start(wt[:], ins[name].rearrange("(t p) e -> p t e", p=P))
            return wt

        def xpose_chunk(src_dram, ktiles, mcc):
            # x [mc-chunk, k] -> x^T chunk [p, kt, m] with k = kt*P + p
            xT = sp.tile([P, ktiles, MC], BF16, tag="xc", bufs=2)
            nc.sync.dma_start_transpose(xT[:], src_dram[mcc * MC:(mcc + 1) * MC, :])
            return xT

        def proj_v(w_sb, src_dram, ktiles):
            # v[m, e] = sum_k x[m, k] w[k, e]  (natural layout, into v_sb)
            for mcc in range(NMC):
                xT = xpose_chunk(src_dram, ktiles, mcc)
                for q2 in range(MC // P):
                    mt = mcc * (MC // P) + q2
                    for ecc in range(NEC):
                        ps = pp.tile([P, EC], F32, tag="pj", bufs=2)
                        for kt in range(ktiles):
                            nc.tensor.matmul(
                                ps[:],
                                xT[:, kt, q2 * P:(q2 + 1) * P],
                                w_sb[:, kt, ecc * EC:(ecc + 1) * EC],
                                start=(kt == 0), stop=(kt == ktiles - 1),
                            )
                        nc.vector.tensor_copy(v_sb[:, mt, ecc * EC:(ecc + 1) * EC], ps[:])

        def proj_T_block(w_sb, ktiles, xT, nt, mcc, sink):
            # out^T[f, m] = sum_k w[k, f] x^T[k, m] for f-tile nt, m-chunk mcc
            ps = pp.tile([P, MC], F32, tag="pj", bufs=2)
            for kt in range(ktiles):
                nc.tensor.matmul(
                    ps[:],
                    w_sb[:, kt, nt * P:(nt + 1) * P],
                    xT[:, kt, :],
                    start=(kt == 0), stop=(kt == ktiles - 1),
                )
            sink(nt, mcc, ps)

        def q_sink(qt_d, bq_tile):
            def sink(nt, mcc, ps):
                qo = sp.tile([P, MC], BF16, tag="qv", bufs=3)
                nc.scalar.activation(qo[:], ps[:], AF.Identity, bias=bq_tile[:, nt:nt + 1])
                nc.gpsimd.dma_start(qt_d[nt * P:(nt + 1) * P, mcc * MC:(mcc + 1) * MC], qo[:])
            return sink

        def k_sink(nt, mcc, ps):
            nc.vector.tensor_copy(kT[:, nt, mcc * MC:(mcc + 1) * MC], ps[:])

        def attention(qt_d, scale, wo_sb, resid_dram, writer):
            for sbi in range(NSB):
                wt_t = sp.tile([P, NI, SB], BF16, tag="wt", bufs=2)
                at_t = sp.tile([P, KB, SB], BF16, tag="at", bufs=2)
                for q3 in range(IPSB):
                    ib = sbi * IPSB + q3
                    qs_t = sp.tile([P, KB, P], BF16, tag="qs", bufs=3)
                    nc.gpsimd.dma_start(
                        qs_t[:],
                        qt_d.rearrange("(t p) m -> p t m", p=P)[:, :, ib * P:(ib + 1) * P],
                    )
                    wb_t = sp.tile([P, n], BF16, tag="wb", bufs=2)
                    ss_t = sp.tile([P, NJH], F32, tag="ss", bufs=2)
                    for jh in range(NJH):
                        ps_s = pp.tile([P, JH], F32, tag="ps_s", bufs=2)
                        for kt in range(KB):
                            for jc in range(NJC):
                                nc.tensor.matmul(
                                    ps_s[:, jc * JC:(jc + 1) * JC],
                                    qs_t[:, kt, :],
                                    kT[:, kt, jh * JH + jc * JC:jh * JH + (jc + 1) * JC],
                                    start=(kt == 0), stop=(kt == KB - 1),
                                )
                        nc.scalar.activation(
                            wb_t[:, jh * JH:(jh + 1) * JH], ps_s[:], AF.Exp,
                            bias=zb[:], scale=scale,
                            accum_out=ss_t[:, jh:jh + 1],
                        )
                    rr = sp.tile([P, 1], F32, tag="rr", bufs=2)
                    if NJH > 1:
                        rs = sp.tile([P, 1], F32, tag="rs", bufs=2)
                        nc.vector.tensor_reduce(rs[:], ss_t[:], axis=mybir.AxisListType.X, op=ALU.add)
                        nc.vector.reciprocal(rr[:], rs[:])
                    else:
                        nc.vector.reciprocal(rr[:], ss_t[:])
                    nc.vector.tensor_scalar_mul(wb_t[:], wb_t[:], rr[:, 0:1])
                    # transpose the normalized weights: w[i, j] -> wT[j, i]
                    wtb = sp.tile([P, NI, P], BF16, tag="wtb", bufs=2)
                    nc.sync.dma_start_transpose(wtb[:], wb_t[:])
                    nc.vector.tensor_copy(wt_t[:, :, q3 * P:(q3 + 1) * P], wtb[:])
                # attn^T[d, i] = sum_j v[j, d] wT[j, i]
                for dt in range(KB):
                    ps_a = pp.tile([P, SB], F32, tag="ps_a", bufs=2)
                    for jt in range(NI):
                        nc.tensor.matmul(
                            ps_a[:],
                            v_sb[:, jt, dt * P:(dt + 1) * P],
                            wt_t[:, jt, :],
                            start=(jt == 0), stop=(jt == NI - 1),
                        )
                    nc.vector.tensor_copy(at_t[:, dt, :], ps_a[:])
                # out-proj + residual
                for q3 in range(IPSB):
                    ib = sbi * IPSB + q3
                    rx = sp.tile([P, db], F32, tag="rx", bufs=2)
                    nc.gpsimd.dma_start(rx[:], resid_dram[ib * P:(ib + 1) * P, :])
                    ro = sp.tile([P, db], F32, tag="ro", bufs=2)
                    for ecc in range(NEC):
                        ps_o = pp.tile([P, EC], F32, tag="pj", bufs=2)
                        for dt in range(KB):
                            nc.tensor.matmul(
                                ps_o[:],
                                at_t[:, dt, q3 * P:(q3 + 1) * P],
                                wo_sb[:, dt, ecc * EC:(ecc + 1) * EC],
                                start=(dt == 0), stop=(dt == KB - 1),
                            )
                        nc.vector.tensor_tensor(
                            ro[:, ecc * EC:(ecc + 1) * EC], ps_o[:],
                            rx[:, ecc * EC:(ecc + 1) * EC], ALU.add,
                        )
                    writer(ib, ro)

        def sa_writer(ib, ro):
            nc.gpsimd.dma_start(xb1_d[ib * P:(ib + 1) * P, :], ro[:])
            rb = sp.tile([P, db], BF16, tag="rb", bufs=2)
            nc.scalar.activation(rb[:], ro[:], AF.Copy)
            nc.gpsimd.dma_start(xb1b_d[ib * P:(ib + 1) * P, :], rb[:])

        def ca_writer(ib, ro):
            nc.gpsimd.dma_start(outs["out"][ib * P:(ib + 1) * P, :], ro[:])

        # ===================== self-attention =====================
        wv = load_w("sa_wv", KB)
        proj_v(wv, ins["xb_bf"], KB)
        wq = load_w("sa_wq", KB)
        wk = load_w("sa_wk", KB)
        sink_q_sa = q_sink(qt_sa_d, bqs)
        for mcc in range(NMC):
            xT = xpose_chunk(ins["xb_bf"], KB, mcc)
            for nt in range(KB):
                proj_T_block(wq, KB, xT, nt, mcc, sink_q_sa)
                proj_T_block(wk, KB, xT, nt, mcc, k_sink)
        wo = load_w("sa_wo", KB)
        attention(qt_sa_d, sc_sa, wo, ins["xbpb"], sa_writer)

        # ===================== cross-attention =====================
        wv2 = load_w("ca_wv", KB)
        proj_v(wv2, xb1b_d, KB)
        wk2 = load_w("ca_wk", KB)
        wq2 = load_w("ca_wq", KA)
        sink_q_ca = q_sink(qt_ca_d, bqc)
        for mcc in range(NMC):
            xTb = xpose_chunk(xb1b_d, KB, mcc)
            for nt in range(KB):
                proj_T_block(wk2, KB, xTb, nt, mcc, k_sink)
            xTa = xpose_chunk(ins["xa_bf"], KA, mcc)
            for nt in range(KB):
                proj_T_block(wq2, KA, xTa, nt, mcc, sink_q_ca)
        wo2 = load_w("ca_wo", KB)
        attention(qt_ca_d, sc_ca, wo2, xb1_d, ca_writer)


def build_program(n=N_FULL, da=DA_FULL, db=DB_FULL):
    """Build the single-core Bass program; returns the Bass module."""
    nc = bacc.Bacc("TRN2", target_bir_lowering=False, debug=False, enable_asserts=False)
    KB = db // P
    ins = {
        "xb_bf": nc.dram_tensor("xb_bf", [n, db], BF16, kind="ExternalInput").ap(),
        "xa_bf": nc.dram_tensor("xa_bf", [n, da], BF16, kind="ExternalInput").ap(),
        "xbpb": nc.dram_tensor("xbpb", [n, db], F32, kind="ExternalInput").ap(),
        "sa_wq": nc.dram_tensor("sa_wq", [db, db], BF16, kind="ExternalInput").ap(),
        "sa_wk": nc.dram_tensor("sa_wk", [db, db], BF16, kind="ExternalInput").ap(),
        "sa_wv": nc.dram_tensor("sa_wv", [db, db], BF16, kind="ExternalInput").ap(),
        "sa_wo": nc.dram_tensor("sa_wo", [db, db], BF16, kind="ExternalInput").ap(),
        "ca_wq": nc.dram_tensor("ca_wq", [da, db], BF16, kind="ExternalInput").ap(),
        "ca_wk": nc.dram_tensor("ca_wk", [db, db], BF16, kind="ExternalInput").ap(),
        "ca_wv": nc.dram_tensor("ca_wv", [db, db], BF16, kind="ExternalInput").ap(),
        "ca_wo": nc.dram_tensor("ca_wo", [db, db], BF16, kind="ExternalInput").ap(),
        "bq_sa": nc.dram_tensor("bq_sa", [P, KB], F32, kind="ExternalInput").ap(),
        "bq_ca": nc.dram_tensor("bq_ca", [P, KB], F32, kind="ExternalInput").ap(),
    }
    outs = {"out": nc.dram_tensor("out", [n, db], F32, kind="ExternalOutput").ap()}
    with tile.TileContext(nc) as tc:
        build_block(tc, outs, ins, n, da, db)
    nc.compile()
    return nc


def prepare_maps(inputs, n=N_FULL, da=DA_FULL, db=DB_FULL):
    """Host-side prep: bf16 casts + exact bias folding.  Returns (in_maps, add_out)."""
    bf = ml_dtypes.bfloat16
    f32 = np.float32
    g = {k: np.ascontiguousarray(np.asarray(v)) for k, v in inputs.items()}
    nb = g["x_a"].shape[0]

    # exact folds (see module docstring); all biases are added in fp32
    b_eff_sa = (g["sa_bv"].astype(f32) @ g["sa_wo"].astype(f32) + g["sa_bo"].astype(f32))
    b_eff_ca = (g["ca_bv"].astype(f32) @ g["ca_wo"].astype(f32) + g["ca_bo"].astype(f32))
    xbpb = (g["x_b"].astype(f32) + b_eff_sa[None, None, :]).astype(f32)

    KB = db // P
    common = {
        "sa_wq": g["sa_wq"].astype(bf), "sa_wk": g["sa_wk"].astype(bf),
        "sa_wv": g["sa_wv"].astype(bf), "sa_wo": g["sa_wo"].astype(bf),
        "ca_wq": g["ca_wq"].astype(bf), "ca_wk": g["ca_wk"].astype(bf),
        "ca_wv": g["ca_wv"].astype(bf), "ca_wo": g["ca_wo"].astype(bf),
        "bq_sa": np.ascontiguousarray(g["sa_bq"].astype(f32).reshape(KB, P).T),
        "bq_ca": np.ascontiguousarray(g["ca_bq"].astype(f32).reshape(KB, P).T),
    }
    in_maps = []
    for b in range(nb):
        in_maps.append(dict(
            xb_bf=g["x_b"][b].astype(bf),
            xa_bf=g["x_a"][b].astype(bf),
            xbpb=np.ascontiguousarray(xbpb[b]),
            **common,
        ))
    return in_maps, b_eff_ca


_CACHE = {}


def run_on_device(inputs, trace=False, **run_kwargs):
    """Run the full problem on 8 NeuronCores.  Returns (out [B,N,DB] f32, results)."""
    if "nc" not in _CACHE:
        _CACHE["nc"] = build_program()
    nc = _CACHE["nc"]
    in_maps, add_out = prepare_maps(inputs)
    res = run_bass_kernel_spmd(
        nc, in_maps, core_ids=list(range(len(in_maps))), trace=trace, **run_kwargs,
    )
    out = np.stack([r["out"] for r in res.results], axis=0)
    out = (out + add_out[None, None, :]).astype(np.float32)
    return out, res


def kernel(**inputs) -> np.ndarray:
    out, _ = run_on_device(inputs)
    return out


# revision 15
# speedup vs baseline: 62.0704x; 62.0704x over previous
"""Trainium2 Bass kernel: dual-attention transformer block (nn_CustomBlock).

Reference semantics (per batch element b):
    q/k/v = x_b @ sa_w{q,k,v} + sa_b{q,k,v}
    sa    = softmax(q k^T / sqrt(DB)) v @ sa_wo + sa_bo
    x_b1  = x_b + sa
    q     = x_a @ ca_wq + ca_bq ; k/v = x_b1 @ ca_w{k,v} + ca_b{k,v}
    out   = x_b1 + softmax(q k^T / sqrt(DA)) v @ ca_wo + ca_bo

Sharding: data-parallel over batch — 8 batch elements, one per NeuronCore,
weights replicated.  No collectives.

Device kernel works in bf16 for all matmul operands (fp32 PSUM accumulation,
fp32 residual stream).  Exact host-side bias folding:
  - k-bias shifts every score row by a constant -> softmax-invariant -> dropped.
  - v-bias passes through attention unchanged (softmax weights sum to 1), so
    bv @ wo + bo folds into a single per-feature vector added to the residual
    input (SA) / the final output (CA) on the host.
  - q-bias is applied on device (per-partition bias in the q^T layout).

Softmax skips the max-subtraction: scores = q.k/sqrt(D) with these operand
scales stays in [-3, 3]; exp() in fp32 is safe by a wide margin.
"""

import math
from contextlib import ExitStack

import numpy as np
import ml_dtypes

import concourse.bass as bass
import concourse.mybir as mybir
import concourse.tile as tile
from concourse import bacc
from concourse.bass_utils import run_bass_kernel_spmd

P = 128
F32 = mybir.dt.float32
BF16 = mybir.dt.bfloat16
AF = mybir.ActivationFunctionType
ALU = mybir.AluOpType

B_FULL, N_FULL, DA_FULL, DB_FULL = 8, 2048, 768, 1024


def build_block(tc, outs, ins, n, da, db):
    """Emit the dual-attention block into TileContext `tc`.

    ins/outs: dicts of DRAM APs:
      ins:  xb_bf [n,db] bf16, xa_bf [n,da] bf16, xbpb [n,db] f32,
            sa_wq/sa_wk/sa_wv/sa_wo [db,db] bf16, ca_wq [da,db] bf16,
            ca_wk/ca_wv/ca_wo [db,db] bf16, bq_sa [P,db/P] f32, bq_ca [P,db/P] f32
      outs: out [n,db] f32
    """
    nc = tc.nc
    KB, KA, NI = db // P, da // P, n // P
    MC = min(512, n)          # projection m-chunk (columns of x^T)
    NMC = n // MC
    JH = min(1024, n)         # scores psum span (2 banks)
    NJH = n // JH
    JC = min(512, JH)         # one psum bank
    NJC = JH // JC
    SB = min(512, n)          # attention superblock (i columns per AV batch)
    NSB = n // SB
    IPSB = SB // P            # i-blocks per superblock
    EC = min(512, db)         # out-proj free chunk
    NEC = db // EC

    sc_sa = 1.0 / math.sqrt(float(db))
    sc_ca = 1.0 / math.sqrt(float(da))

    ctx = ExitStack()
    with ctx:
        sp = ctx.enter_context(tc.tile_pool(name="sp", bufs=1))
        pp = ctx.enter_context(tc.tile_pool(name="pp", bufs=1, space="PSUM"))
        dp = ctx.enter_context(tc.tile_pool(name="dp", bufs=1, space="DRAM"))

        # DRAM scratch
        qt_sa_d = dp.tile([db, n], BF16, tag="qt_sa")
        qt_ca_d = dp.tile([db, n], BF16, tag="qt_ca")
        xb1_d = dp.tile([n, db], F32, tag="xb1")
        xb1b_d = dp.tile([n, db], BF16, tag="xb1b")

        # persistent SBUF
        kT = sp.tile([P, KB, n], BF16, tag="kT")        # k^T  [feat, seq]
        v_sb = sp.tile([P, NI, db], BF16, tag="v")      # v    [seq, feat]
        bqs = sp.tile([P, KB], F32, tag="bqs")
        bqc = sp.tile([P, KB], F32, tag="bqc")
        zb = sp.tile([P, 1], F32, tag="zb")
        nc.sync.dma_start(bqs[:], ins["bq_sa"][:])
        nc.sync.dma_start(bqc[:], ins["bq_ca"][:])
        nc.gpsimd.memset(zb[:], 0.0)

        def load_w(name, ktiles):
            wt = sp.tile([P, ktiles, db], BF16, tag="w", bufs=2)
            nc.sync.dma_start(wt[:], ins[name].rearrange("(t p) e -> p t e", p=P))
            return wt

        def xpose_chunk(src_dram, ktiles, mcc):
            # x [mc-chunk, k] -> x^T chunk [p, kt, m] with k = kt*P + p
            # (tag shared with the attention wT superblock tiles: the phases
            # are sequential, and sharing keeps total SBUF under the cap)
            xT = sp.tile([P, ktiles, MC], BF16, tag="xcwt", bufs=2)
            nc.sync.dma_start_transpose(xT[:], src_dram[mcc * MC:(mcc + 1) * MC, :])
            return xT

        def proj_v(w_sb, src_dram, ktiles):
            # v[m, e] = sum_k x[m, k] w[k, e]  (natural layout, into v_sb)
            for mcc in range(NMC):
                xT = xpose_chunk(src_dram, ktiles, mcc)
                for q2 in range(MC // P):
                    mt = mcc * (MC // P) + q2
                    for ecc in range(NEC):
                        ps = pp.tile([P, EC], F32, tag="pj", bufs=2)
                        for kt in range(ktiles):
                            nc.tensor.matmul(
                                ps[:],
                                xT[:, kt, q2 * P:(q2 + 1) * P],
                                w_sb[:, kt, ecc * EC:(ecc + 1) * EC],
                                start=(kt == 0), stop=(kt == ktiles - 1),
                            )
                        nc.vector.tensor_copy(v_sb[:, mt, ecc * EC:(ecc + 1) * EC], ps[:])

        def proj_T_block(w_sb, ktiles, xT, nt, mcc, sink):
            # out^T[f, m] = sum_k w[k, f] x^T[k, m] for f-tile nt, m-chunk mcc
            ps = pp.tile([P, MC], F32, tag="pj", bufs=2)
            for kt in range(ktiles):
                nc.tensor.matmul(
                    ps[:],
                    w_sb[:, kt, nt * P:(nt + 1) * P],
                    xT[:, kt, :],
                    start=(kt == 0), stop=(kt == ktiles - 1),
                )
            sink(nt, mcc, ps)

        def q_sink(qt_d, bq_tile):
            def sink(nt, mcc, ps):
                qo = sp.tile([P, MC], BF16, tag="qv", bufs=2)
                nc.scalar.activation(qo[:], ps[:], AF.Identity, bias=bq_tile[:, nt:nt + 1])
                nc.sync.dma_start(qt_d[nt * P:(nt + 1) * P, mcc * MC:(mcc + 1) * MC], qo[:])
            return sink

        def k_sink(nt, mcc, ps):
            nc.vector.tensor_copy(kT[:, nt, mcc * MC:(mcc + 1) * MC], ps[:])

        def attention(qt_d, scale, wo_sb, resid_dram, writer):
            # Software-pipelined over superblocks: the scores/softmax/transpose
            # chain of superblock sbi is emitted BEFORE the AV/out-proj of
            # sbi-1, so the PE never stalls on the (ACT/DVE/DMA) softmax tail.
            def scores_phase(sbi, wt_t):
                for q3 in range(IPSB):
                    ib = sbi * IPSB + q3
                    qs_t = sp.tile([P, KB, P], BF16, tag="qs", bufs=2)
                    nc.sync.dma_start(
                        qs_t[:],
                        qt_d.rearrange("(t p) m -> p t m", p=P)[:, :, ib * P:(ib + 1) * P],
                    )
                    wb_t = sp.tile([P, n], BF16, tag="wb", bufs=2)
                    ss_t = sp.tile([P, NJH], F32, tag="ss", bufs=2)
                    for jh in range(NJH):
                        ps_s = pp.tile([P, JH], F32, tag="ps_s", bufs=2)
                        for kt in range(KB):
                            for jc in range(NJC):
                                nc.tensor.matmul(
                                    ps_s[:, jc * JC:(jc + 1) * JC],
                                    qs_t[:, kt, :],
                                    kT[:, kt, jh * JH + jc * JC:jh * JH + (jc + 1) * JC],
                                    start=(kt == 0), stop=(kt == KB - 1),
                                )
                        nc.scalar.activation(
                            wb_t[:, jh * JH:(jh + 1) * JH], ps_s[:], AF.Exp,
                            bias=zb[:], scale=scale,
                            accum_out=ss_t[:, jh:jh + 1],
                        )
                    rr = sp.tile([P, 1], F32, tag="rr", bufs=2)
                    if NJH > 1:
                        rs = sp.tile([P, 1], F32, tag="rs", bufs=2)
                        nc.vector.tensor_reduce(rs[:], ss_t[:], axis=mybir.AxisListType.X, op=ALU.add)
                        nc.vector.reciprocal(rr[:], rs[:])
                    else:
                        nc.vector.reciprocal(rr[:], ss_t[:])
                    nc.vector.tensor_scalar_mul(wb_t[:], wb_t[:], rr[:, 0:1])
                    # transpose the normalized weights: w[i, j] -> wT[j, i]
                    wtb = sp.tile([P, NI, P], BF16, tag="wtb", bufs=2)
                    nc.sync.dma_start_transpose(wtb[:], wb_t[:])
                    nc.vector.tensor_copy(wt_t[:, :, q3 * P:(q3 + 1) * P], wtb[:])

            def av_part(sbi, wt_t):
                # attn^T[d, i] = sum_j v[j, d] wT[j, i]
                at_t = sp.tile([P, KB, SB], BF16, tag="at", bufs=2)
                for dt in range(KB):
                    ps_a = pp.tile([P, SB], F32, tag="ps_a", bufs=2)
                    for jt in range(NI):
                        nc.tensor.matmul(
                            ps_a[:],
                            v_sb[:, jt, dt * P:(dt + 1) * P],
                            wt_t[:, jt, :],
                            start=(jt == 0), stop=(jt == NI - 1),
                        )
                    nc.vector.tensor_copy(at_t[:, dt, :], ps_a[:])
                return at_t

            def op_part(sbi, at_t):
                # out-proj + residual
                for q3 in range(IPSB):
                    ib = sbi * IPSB + q3
                    rx = sp.tile([P, db], F32, tag="rx", bufs=2)
                    nc.sync.dma_start(rx[:], resid_dram[ib * P:(ib + 1) * P, :])
                    ro = sp.tile([P, db], F32, tag="ro", bufs=2)
                    for ecc in range(NEC):
                        ps_o = pp.tile([P, EC], F32, tag="pj", bufs=2)
                        for dt in range(KB):
                            nc.tensor.matmul(
                                ps_o[:],
                                at_t[:, dt, q3 * P:(q3 + 1) * P],
                                wo_sb[:, dt, ecc * EC:(ecc + 1) * EC],
                                start=(dt == 0), stop=(dt == KB - 1),
                            )
                        nc.vector.tensor_tensor(
                            ro[:, ecc * EC:(ecc + 1) * EC], ps_o[:],
                            rx[:, ecc * EC:(ecc + 1) * EC], ALU.add,
                        )
                    writer(ib, ro)

            pend_av = None   # (sbi, wt_t) awaiting AV
            pend_op = None   # (sbi, at_t) awaiting out-proj
            for sbi in range(NSB):
                wt_t = sp.tile([P, NI, SB], BF16, tag="xcwt", bufs=2)
                scores_phase(sbi, wt_t)
                new_at = av_part(*pend_av) if pend_av is not None else None
                if pend_op is not None:
                    op_part(*pend_op)
                if new_at is not None:
                    pend_op = (pend_av[0], new_at)
                pend_av = (sbi, wt_t)
            at_t = av_part(*pend_av)
            if pend_op is not None:
                op_part(*pend_op)
            op_part(pend_av[0], at_t)

        def sa_writer(ib, ro):
            nc.sync.dma_start(xb1_d[ib * P:(ib + 1) * P, :], ro[:])
            rb = sp.tile([P, db], BF16, tag="rb", bufs=2)
            nc.scalar.activation(rb[:], ro[:], AF.Copy)
            nc.sync.dma_start(xb1b_d[ib * P:(ib + 1) * P, :], rb[:])

        def ca_writer(ib, ro):
            nc.sync.dma_start(outs["out"][ib * P:(ib + 1) * P, :], ro[:])

        # ===================== self-attention =====================
        wv = load_w("sa_wv", KB)
        proj_v(wv, ins["xb_bf"], KB)
        wq = load_w("sa_wq", KB)
        wk = load_w("sa_wk", KB)
        sink_q_sa = q_sink(qt_sa_d, bqs)
        for mcc in range(NMC):
            xT = xpose_chunk(ins["xb_bf"], KB, mcc)
            for nt in range(KB):
                proj_T_block(wq, KB, xT, nt, mcc, sink_q_sa)
                proj_T_block(wk, KB, xT, nt, mcc, k_sink)
        wo = load_w("sa_wo", KB)
        attention(qt_sa_d, sc_sa, wo, ins["xbpb"], sa_writer)

        # ===================== cross-attention =====================
        wv2 = load_w("ca_wv", KB)
        proj_v(wv2, xb1b_d, KB)
        wk2 = load_w("ca_wk", KB)
        sink_q_ca = q_sink(qt_ca_d, bqc)
        for mcc in range(NMC):
            xTb = xpose_chunk(xb1b_d, KB, mcc)
            for nt in range(KB):
                proj_T_block(wk2, KB, xTb, nt, mcc, k_sink)
        wq2 = load_w("ca_wq", KA)
        for mcc in range(NMC):
            xTa = xpose_chunk(ins["xa_bf"], KA, mcc)
            for nt in range(KB):
                proj_T_block(wq2, KA, xTa, nt, mcc, sink_q_ca)
        wo2 = load_w("ca_wo", KB)
        attention(qt_ca_d, sc_ca, wo2, xb1_d, ca_writer)


def build_program(n=N_FULL, da=DA_FULL, db=DB_FULL, repeat=1):
    """Build the single-core Bass program; returns the Bass module.

    repeat>1 re-emits the whole block body N times (idempotent — same inputs
    and scratch): used to measure per-iteration device time above the fixed
    dispatch overhead."""
    nc = bacc.Bacc("TRN2", target_bir_lowering=False, debug=False, enable_asserts=False)
    KB = db // P
    ins = {
        "xb_bf": nc.dram_tensor("xb_bf", [n, db], BF16, kind="ExternalInput").ap(),
        "xa_bf": nc.dram_tensor("xa_bf", [n, da], BF16, kind="ExternalInput").ap(),
        "xbpb": nc.dram_tensor("xbpb", [n, db], F32, kind="ExternalInput").ap(),
        "sa_wq": nc.dram_tensor("sa_wq", [db, db], BF16, kind="ExternalInput").ap(),
        "sa_wk": nc.dram_tensor("sa_wk", [db, db], BF16, kind="ExternalInput").ap(),
        "sa_wv": nc.dram_tensor("sa_wv", [db, db], BF16, kind="ExternalInput").ap(),
        "sa_wo": nc.dram_tensor("sa_wo", [db, db], BF16, kind="ExternalInput").ap(),
        "ca_wq": nc.dram_tensor("ca_wq", [da, db], BF16, kind="ExternalInput").ap(),
        "ca_wk": nc.dram_tensor("ca_wk", [db, db], BF16, kind="ExternalInput").ap(),
        "ca_wv": nc.dram_tensor("ca_wv", [db, db], BF16, kind="ExternalInput").ap(),
        "ca_wo": nc.dram_tensor("ca_wo", [db, db], BF16, kind="ExternalInput").ap(),
        "bq_sa": nc.dram_tensor("bq_sa", [P, KB], F32, kind="ExternalInput").ap(),
        "bq_ca": nc.dram_tensor("bq_ca", [P, KB], F32, kind="ExternalInput").ap(),
    }
    outs = {"out": nc.dram_tensor("out", [n, db], F32, kind="ExternalOutput").ap()}
    with tile.TileContext(nc) as tc:
        for _ in range(repeat):
            build_block(tc, outs, ins, n, da, db)
    nc.compile()
    return nc


def prepare_maps(inputs, n=N_FULL, da=DA_FULL, db=DB_FULL):
    """Host-side prep: bf16 casts + exact bias folding.  Returns (in_maps, add_out)."""
    bf = ml_dtypes.bfloat16
    f32 = np.float32
    g = {k: np.ascontiguousarray(np.asarray(v)) for k, v in inputs.items()}
    nb = g["x_a"].shape[0]

    # exact folds (see module docstring); all biases are added in fp32
    b_eff_sa = (g["sa_bv"].astype(f32) @ g["sa_wo"].astype(f32) + g["sa_bo"].astype(f32))
    b_eff_ca = (g["ca_bv"].astype(f32) @ g["ca_wo"].astype(f32) + g["ca_bo"].astype(f32))
    xbpb = (g["x_b"].astype(f32) + b_eff_sa[None, None, :]).astype(f32)

    KB = db // P
    common = {
        "sa_wq": g["sa_wq"].astype(bf), "sa_wk": g["sa_wk"].astype(bf),
        "sa_wv": g["sa_wv"].astype(bf), "sa_wo": g["sa_wo"].astype(bf),
        "ca_wq": g["ca_wq"].astype(bf), "ca_wk": g["ca_wk"].astype(bf),
        "ca_wv": g["ca_wv"].astype(bf), "ca_wo": g["ca_wo"].astype(bf),
        "bq_sa": np.ascontiguousarray(g["sa_bq"].astype(f32).reshape(KB, P).T),
        "bq_ca": np.ascontiguousarray(g["ca_bq"].astype(f32).reshape(KB, P).T),
    }
    in_maps = []
    for b in range(nb):
        in_maps.append(dict(
            xb_bf=g["x_b"][b].astype(bf),
            xa_bf=g["x_a"][b].astype(bf),
            xbpb=np.ascontiguousarray(xbpb[b]),
            **common,
        ))
    return in_maps, b_eff_ca


_CACHE = {}


def run_on_device(inputs, trace=False, **run_kwargs):
    """Run the full problem on 8 NeuronCores.  Returns (out [B,N,DB] f32, results)."""
    if "nc" not in _CACHE:
        _CACHE["nc"] = build_program()
    nc = _CACHE["nc"]
    in_maps, add_out = prepare_maps(inputs)
    res = run_bass_kernel_spmd(
        nc, in_maps, core_ids=list(range(len(in_maps))), trace=trace, **run_kwargs,
    )
    out = np.stack([r["out"] for r in res.results], axis=0)
    out = (out + add_out[None, None, :]).astype(np.float32)
    return out, res


def kernel(**inputs) -> np.ndarray:
    out, _ = run_on_device(inputs)
    return out


# revision 17
# speedup vs baseline: 64.5826x; 1.0405x over previous
"""Trainium2 Bass kernel: dual-attention transformer block (nn_CustomBlock).

Reference semantics (per batch element b):
    q/k/v = x_b @ sa_w{q,k,v} + sa_b{q,k,v}
    sa    = softmax(q k^T / sqrt(DB)) v @ sa_wo + sa_bo
    x_b1  = x_b + sa
    q     = x_a @ ca_wq + ca_bq ; k/v = x_b1 @ ca_w{k,v} + ca_b{k,v}
    out   = x_b1 + softmax(q k^T / sqrt(DA)) v @ ca_wo + ca_bo

Sharding: data-parallel over batch — 8 batch elements, one per NeuronCore,
weights replicated.  No collectives.

Device kernel works in bf16 for all matmul operands (fp32 PSUM accumulation,
fp32 residual stream).  Exact host-side bias folding:
  - k-bias shifts every score row by a constant -> softmax-invariant -> dropped.
  - v-bias passes through attention unchanged (softmax weights sum to 1), so
    bv @ wo + bo folds into a single per-feature vector added to the residual
    input (SA) / the final output (CA) on the host.
  - q-bias is applied on device (per-partition bias in the q^T layout).

Softmax skips the max-subtraction: scores = q.k/sqrt(D) with these operand
scales stays in [-3, 3]; exp() in fp32 is safe by a wide margin.
"""

import math
import os
from contextlib import ExitStack

import numpy as np
import ml_dtypes

import concourse.bass as bass
import concourse.mybir as mybir
import concourse.tile as tile
from concourse import bacc
from concourse.bass_utils import run_bass_kernel_spmd

P = 128
F32 = mybir.dt.float32
BF16 = mybir.dt.bfloat16
AF = mybir.ActivationFunctionType
ALU = mybir.AluOpType

B_FULL, N_FULL, DA_FULL, DB_FULL = 8, 2048, 768, 1024


def build_block(tc, outs, ins, n, da, db):
    """Emit the dual-attention block into TileContext `tc`.

    ins/outs: dicts of DRAM APs:
      ins:  xb_bf [n,db] bf16, xa_bf [n,da] bf16, xbpb [n,db] f32,
            sa_wq/sa_wk/sa_wv/sa_wo [db,db] bf16, ca_wq [da,db] bf16,
            ca_wk/ca_wv/ca_wo [db,db] bf16, bq_sa [P,db/P] f32, bq_ca [P,db/P] f32
      outs: out [n,db] f32
    """
    nc = tc.nc
    KB, KA, NI = db // P, da // P, n // P
    MC = min(512, n)          # projection m-chunk (columns of x^T)
    NMC = n // MC
    JH = min(1024, n)         # scores psum span (2 banks)
    NJH = n // JH
    JC = min(512, JH)         # one psum bank
    NJC = JH // JC
    SB = min(512, n)          # attention superblock (i columns per AV batch)
    NSB = n // SB
    IPSB = SB // P            # i-blocks per superblock
    EC = min(512, db)         # out-proj free chunk
    NEC = db // EC

    sc_sa = 1.0 / math.sqrt(float(db))
    sc_ca = 1.0 / math.sqrt(float(da))

    ctx = ExitStack()
    with ctx:
        sp = ctx.enter_context(tc.tile_pool(name="sp", bufs=1))
        pp = ctx.enter_context(tc.tile_pool(name="pp", bufs=1, space="PSUM"))
        dp = ctx.enter_context(tc.tile_pool(name="dp", bufs=1, space="DRAM"))

        # DRAM scratch
        qt_sa_d = dp.tile([db, n], BF16, tag="qt_sa")
        qt_ca_d = dp.tile([db, n], BF16, tag="qt_ca")
        xb1_d = dp.tile([n, db], F32, tag="xb1")
        xb1b_d = dp.tile([n, db], BF16, tag="xb1b")

        # persistent SBUF
        kT = sp.tile([P, KB, n], BF16, tag="kT")        # k^T  [feat, seq]
        v_sb = sp.tile([P, NI, db], BF16, tag="v")      # v    [seq, feat]
        bqs = sp.tile([P, KB], F32, tag="bqs")
        bqc = sp.tile([P, KB], F32, tag="bqc")
        zb = sp.tile([P, 1], F32, tag="zb")
        nc.sync.dma_start(bqs[:], ins["bq_sa"][:])
        nc.sync.dma_start(bqc[:], ins["bq_ca"][:])
        nc.gpsimd.memset(zb[:], 0.0)

        def load_w(name, ktiles):
            wt = sp.tile([P, ktiles, db], BF16, tag="w", bufs=2)
            nc.sync.dma_start(wt[:], ins[name].rearrange("(t p) e -> p t e", p=P))
            return wt

        def xpose_chunk(src_dram, ktiles, mcc):
            # x [mc-chunk, k] -> x^T chunk [p, kt, m] with k = kt*P + p
            # (tag shared with the attention wT superblock tiles: the phases
            # are sequential, and sharing keeps total SBUF under the cap)
            xT = sp.tile([P, ktiles, MC], BF16, tag="xcwt", bufs=2)
            nc.sync.dma_start_transpose(xT[:], src_dram[mcc * MC:(mcc + 1) * MC, :])
            return xT

        def proj_v(w_sb, src_dram, ktiles):
            # v[m, e] = sum_k x[m, k] w[k, e]  (natural layout, into v_sb)
            for mcc in range(NMC):
                xT = xpose_chunk(src_dram, ktiles, mcc)
                for q2 in range(MC // P):
                    mt = mcc * (MC // P) + q2
                    for ecc in range(NEC):
                        ps = pp.tile([P, EC], F32, tag="pj", bufs=2)
                        for kt in range(ktiles):
                            nc.tensor.matmul(
                                ps[:],
                                xT[:, kt, q2 * P:(q2 + 1) * P],
                                w_sb[:, kt, ecc * EC:(ecc + 1) * EC],
                                start=(kt == 0), stop=(kt == ktiles - 1),
                            )
                        nc.vector.tensor_copy(v_sb[:, mt, ecc * EC:(ecc + 1) * EC], ps[:])

        def proj_T_block(w_sb, ktiles, xT, nt, mcc, sink):
            # out^T[f, m] = sum_k w[k, f] x^T[k, m] for f-tile nt, m-chunk mcc
            ps = pp.tile([P, MC], F32, tag="pj", bufs=2)
            for kt in range(ktiles):
                nc.tensor.matmul(
                    ps[:],
                    w_sb[:, kt, nt * P:(nt + 1) * P],
                    xT[:, kt, :],
                    start=(kt == 0), stop=(kt == ktiles - 1),
                )
            sink(nt, mcc, ps)

        def q_sink(qt_d, bq_tile):
            def sink(nt, mcc, ps):
                qo = sp.tile([P, MC], BF16, tag="qv", bufs=2)
                nc.scalar.activation(qo[:], ps[:], AF.Identity, bias=bq_tile[:, nt:nt + 1])
                nc.sync.dma_start(qt_d[nt * P:(nt + 1) * P, mcc * MC:(mcc + 1) * MC], qo[:])
            return sink

        def k_sink(nt, mcc, ps):
            nc.vector.tensor_copy(kT[:, nt, mcc * MC:(mcc + 1) * MC], ps[:])

        def attention(qt_d, scale, wo_sb, resid_dram, writer):
            # Software-pipelined over superblocks: the scores/softmax/transpose
            # chain of superblock sbi is emitted BEFORE the AV/out-proj of
            # sbi-1, so the PE never stalls on the (ACT/DVE/DMA) softmax tail.
            def scores_phase(sbi, wt_t):
                for q3 in range(IPSB):
                    ib = sbi * IPSB + q3
                    qs_t = sp.tile([P, KB, P], BF16, tag="qs", bufs=2)
                    nc.sync.dma_start(
                        qs_t[:],
                        qt_d.rearrange("(t p) m -> p t m", p=P)[:, :, ib * P:(ib + 1) * P],
                    )
                    wb_t = sp.tile([P, n], BF16, tag="wb", bufs=2)
                    ss_t = sp.tile([P, NJH], F32, tag="ss", bufs=2)
                    for jh in range(NJH):
                        ps_s = pp.tile([P, JH], F32, tag="ps_s", bufs=2)
                        for kt in range(KB):
                            for jc in range(NJC):
                                nc.tensor.matmul(
                                    ps_s[:, jc * JC:(jc + 1) * JC],
                                    qs_t[:, kt, :],
                                    kT[:, kt, jh * JH + jc * JC:jh * JH + (jc + 1) * JC],
                                    start=(kt == 0), stop=(kt == KB - 1),
                                )
                        nc.scalar.activation(
                            wb_t[:, jh * JH:(jh + 1) * JH], ps_s[:], AF.Exp,
                            bias=zb[:], scale=scale,
                            accum_out=ss_t[:, jh:jh + 1],
                        )
                    rr = sp.tile([P, 1], F32, tag="rr", bufs=2)
                    if NJH > 1:
                        rs = sp.tile([P, 1], F32, tag="rs", bufs=2)
                        nc.vector.tensor_reduce(rs[:], ss_t[:], axis=mybir.AxisListType.X, op=ALU.add)
                        nc.vector.reciprocal(rr[:], rs[:])
                    else:
                        nc.vector.reciprocal(rr[:], ss_t[:])
                    nc.vector.tensor_scalar_mul(wb_t[:], wb_t[:], rr[:, 0:1])
                    # transpose the normalized weights: w[i, j] -> wT[j, i]
                    wtb = sp.tile([P, NI, P], BF16, tag="wtb", bufs=2)
                    nc.sync.dma_start_transpose(wtb[:], wb_t[:])
                    nc.vector.tensor_copy(wt_t[:, :, q3 * P:(q3 + 1) * P], wtb[:])

            def av_part(sbi, wt_t):
                # attn^T[d, i] = sum_j v[j, d] wT[j, i]
                at_t = sp.tile([P, KB, SB], BF16, tag="at", bufs=2)
                for dt in range(KB):
                    ps_a = pp.tile([P, SB], F32, tag="ps_a", bufs=2)
                    for jt in range(NI):
                        nc.tensor.matmul(
                            ps_a[:],
                            v_sb[:, jt, dt * P:(dt + 1) * P],
                            wt_t[:, jt, :],
                            start=(jt == 0), stop=(jt == NI - 1),
                        )
                    nc.vector.tensor_copy(at_t[:, dt, :], ps_a[:])
                return at_t

            def op_part(sbi, at_t):
                # out-proj + residual
                for q3 in range(IPSB):
                    ib = sbi * IPSB + q3
                    rx = sp.tile([P, db], F32, tag="rx", bufs=2)
                    nc.sync.dma_start(rx[:], resid_dram[ib * P:(ib + 1) * P, :])
                    ro = sp.tile([P, db], F32, tag="ro", bufs=2)
                    for ecc in range(NEC):
                        ps_o = pp.tile([P, EC], F32, tag="pj", bufs=2)
                        for dt in range(KB):
                            nc.tensor.matmul(
                                ps_o[:],
                                at_t[:, dt, q3 * P:(q3 + 1) * P],
                                wo_sb[:, dt, ecc * EC:(ecc + 1) * EC],
                                start=(dt == 0), stop=(dt == KB - 1),
                            )
                        nc.vector.tensor_tensor(
                            ro[:, ecc * EC:(ecc + 1) * EC], ps_o[:],
                            rx[:, ecc * EC:(ecc + 1) * EC], ALU.add,
                        )
                    writer(ib, ro)

            pend_av = None   # (sbi, wt_t) awaiting AV
            pend_op = None   # (sbi, at_t) awaiting out-proj
            for sbi in range(NSB):
                wt_t = sp.tile([P, NI, SB], BF16, tag="xcwt", bufs=2)
                scores_phase(sbi, wt_t)
                new_at = av_part(*pend_av) if pend_av is not None else None
                if pend_op is not None:
                    op_part(*pend_op)
                if new_at is not None:
                    pend_op = (pend_av[0], new_at)
                pend_av = (sbi, wt_t)
            at_t = av_part(*pend_av)
            if pend_op is not None:
                op_part(*pend_op)
            op_part(pend_av[0], at_t)

        def sa_writer(ib, ro):
            nc.sync.dma_start(xb1_d[ib * P:(ib + 1) * P, :], ro[:])
            rb = sp.tile([P, db], BF16, tag="rb", bufs=2)
            nc.scalar.activation(rb[:], ro[:], AF.Copy)
            nc.sync.dma_start(xb1b_d[ib * P:(ib + 1) * P, :], rb[:])

        def ca_writer(ib, ro):
            nc.sync.dma_start(outs["out"][ib * P:(ib + 1) * P, :], ro[:])

        # ===================== self-attention =====================
        wv = load_w("sa_wv", KB)
        proj_v(wv, ins["xb_bf"], KB)
        wq = load_w("sa_wq", KB)
        wk = load_w("sa_wk", KB)
        sink_q_sa = q_sink(qt_sa_d, bqs)
        for mcc in range(NMC):
            xT = xpose_chunk(ins["xb_bf"], KB, mcc)
            for nt in range(KB):
                proj_T_block(wq, KB, xT, nt, mcc, sink_q_sa)
                proj_T_block(wk, KB, xT, nt, mcc, k_sink)
        wo = load_w("sa_wo", KB)
        attention(qt_sa_d, sc_sa, wo, ins["xbpb"], sa_writer)

        # ===================== cross-attention =====================
        wv2 = load_w("ca_wv", KB)
        proj_v(wv2, xb1b_d, KB)
        wk2 = load_w("ca_wk", KB)
        sink_q_ca = q_sink(qt_ca_d, bqc)
        for mcc in range(NMC):
            xTb = xpose_chunk(xb1b_d, KB, mcc)
            for nt in range(KB):
                proj_T_block(wk2, KB, xTb, nt, mcc, k_sink)
        wq2 = load_w("ca_wq", KA)
        for mcc in range(NMC):
            xTa = xpose_chunk(ins["xa_bf"], KA, mcc)
            for nt in range(KB):
                proj_T_block(wq2, KA, xTa, nt, mcc, sink_q_ca)
        wo2 = load_w("ca_wo", KB)
        attention(qt_ca_d, sc_ca, wo2, xb1_d, ca_writer)


def build_program(n=N_FULL, da=DA_FULL, db=DB_FULL, repeat=1):
    """Build the single-core Bass program; returns the Bass module.

    repeat>1 re-emits the whole block body N times (idempotent — same inputs
    and scratch): used to measure per-iteration device time above the fixed
    dispatch overhead."""
    nc = bacc.Bacc("TRN2", target_bir_lowering=False, debug=False, enable_asserts=False)
    KB = db // P
    ins = {
        "xb_bf": nc.dram_tensor("xb_bf", [n, db], BF16, kind="ExternalInput").ap(),
        "xa_bf": nc.dram_tensor("xa_bf", [n, da], BF16, kind="ExternalInput").ap(),
        "xbpb": nc.dram_tensor("xbpb", [n, db], F32, kind="ExternalInput").ap(),
        "sa_wq": nc.dram_tensor("sa_wq", [db, db], BF16, kind="ExternalInput").ap(),
        "sa_wk": nc.dram_tensor("sa_wk", [db, db], BF16, kind="ExternalInput").ap(),
        "sa_wv": nc.dram_tensor("sa_wv", [db, db], BF16, kind="ExternalInput").ap(),
        "sa_wo": nc.dram_tensor("sa_wo", [db, db], BF16, kind="ExternalInput").ap(),
        "ca_wq": nc.dram_tensor("ca_wq", [da, db], BF16, kind="ExternalInput").ap(),
        "ca_wk": nc.dram_tensor("ca_wk", [db, db], BF16, kind="ExternalInput").ap(),
        "ca_wv": nc.dram_tensor("ca_wv", [db, db], BF16, kind="ExternalInput").ap(),
        "ca_wo": nc.dram_tensor("ca_wo", [db, db], BF16, kind="ExternalInput").ap(),
        "bq_sa": nc.dram_tensor("bq_sa", [P, KB], F32, kind="ExternalInput").ap(),
        "bq_ca": nc.dram_tensor("bq_ca", [P, KB], F32, kind="ExternalInput").ap(),
    }
    outs = {"out": nc.dram_tensor("out", [n, db], F32, kind="ExternalOutput").ap()}
    with tile.TileContext(nc) as tc:
        for _ in range(repeat):
            build_block(tc, outs, ins, n, da, db)
    nc.compile()
    return nc


def prepare_maps(inputs, n=N_FULL, da=DA_FULL, db=DB_FULL):
    """Host-side prep: bf16 casts + exact bias folding.  Returns (in_maps, add_out)."""
    bf = ml_dtypes.bfloat16
    f32 = np.float32
    g = {k: np.ascontiguousarray(np.asarray(v)) for k, v in inputs.items()}
    nb = g["x_a"].shape[0]

    # exact folds (see module docstring); all biases are added in fp32
    b_eff_sa = (g["sa_bv"].astype(f32) @ g["sa_wo"].astype(f32) + g["sa_bo"].astype(f32))
    b_eff_ca = (g["ca_bv"].astype(f32) @ g["ca_wo"].astype(f32) + g["ca_bo"].astype(f32))
    xbpb = (g["x_b"].astype(f32) + b_eff_sa[None, None, :]).astype(f32)

    KB = db // P
    common = {
        "sa_wq": g["sa_wq"].astype(bf), "sa_wk": g["sa_wk"].astype(bf),
        "sa_wv": g["sa_wv"].astype(bf), "sa_wo": g["sa_wo"].astype(bf),
        "ca_wq": g["ca_wq"].astype(bf), "ca_wk": g["ca_wk"].astype(bf),
        "ca_wv": g["ca_wv"].astype(bf), "ca_wo": g["ca_wo"].astype(bf),
        "bq_sa": np.ascontiguousarray(g["sa_bq"].astype(f32).reshape(KB, P).T),
        "bq_ca": np.ascontiguousarray(g["ca_bq"].astype(f32).reshape(KB, P).T),
    }
    in_maps = []
    for b in range(nb):
        in_maps.append(dict(
            xb_bf=g["x_b"][b].astype(bf),
            xa_bf=g["x_a"][b].astype(bf),
            xbpb=np.ascontiguousarray(xbpb[b]),
            **common,
        ))
    return in_maps, b_eff_ca


_CACHE = {}


def run_on_device(inputs, trace=False, **run_kwargs):
    """Run the full problem on 8 NeuronCores.  Returns (out [B,N,DB] f32, results)."""
    if not trace:
        # NTFF tracing needs antenv.axon_hooks, absent in this container; make
        # sure an inherited BASS_TRACE=1 can't route us into that path.
        os.environ.setdefault("BASS_NEVER_TRACE", "1")
    if "nc" not in _CACHE:
        _CACHE["nc"] = build_program()
    nc = _CACHE["nc"]
    in_maps, add_out = prepare_maps(inputs)
    res = run_bass_kernel_spmd(
        nc, in_maps, core_ids=list(range(len(in_maps))), trace=trace, **run_kwargs,
    )
    out = np.stack([r["out"] for r in res.results], axis=0)
    out = (out + add_out[None, None, :]).astype(np.float32)
    return out, res


def kernel(**inputs) -> np.ndarray:
    out, _ = run_on_device(inputs)
    return out


# revision 21
# speedup vs baseline: 74.6424x; 1.1558x over previous
"""Trainium2 Bass kernel: dual-attention transformer block (nn_CustomBlock).

Reference semantics (per batch element b):
    q/k/v = x_b @ sa_w{q,k,v} + sa_b{q,k,v}
    sa    = softmax(q k^T / sqrt(DB)) v @ sa_wo + sa_bo
    x_b1  = x_b + sa
    q     = x_a @ ca_wq + ca_bq ; k/v = x_b1 @ ca_w{k,v} + ca_b{k,v}
    out   = x_b1 + softmax(q k^T / sqrt(DA)) v @ ca_wo + ca_bo

Sharding: data-parallel over batch — 8 batch elements, one per NeuronCore,
weights replicated.  No collectives.

Device kernel works in bf16 for all matmul operands (fp32 PSUM accumulation,
fp32 residual stream).  Exact host-side bias folding:
  - k-bias shifts every score row by a constant -> softmax-invariant -> dropped.
  - v-bias passes through attention unchanged (softmax weights sum to 1), so
    bv @ wo + bo folds into a single per-feature vector added to the residual
    input (SA) / the final output (CA) on the host.
  - q-bias is applied on device (per-partition bias in the q^T layout).

Softmax skips the max-subtraction: scores = q.k/sqrt(D) with these operand
scales stays in [-3, 3]; exp() in fp32 is safe by a wide margin.
"""

import math
import os
from contextlib import ExitStack

import numpy as np
import ml_dtypes

import concourse.bass as bass
import concourse.mybir as mybir
import concourse.tile as tile
from concourse import bacc
from concourse.bass_utils import run_bass_kernel_spmd

P = 128
F32 = mybir.dt.float32
BF16 = mybir.dt.bfloat16
AF = mybir.ActivationFunctionType
ALU = mybir.AluOpType

B_FULL, N_FULL, DA_FULL, DB_FULL = 8, 2048, 768, 1024


def build_block(tc, outs, ins, n, da, db):
    """Emit the dual-attention block into TileContext `tc`.

    ins/outs: dicts of DRAM APs:
      ins:  xb_bf [n,db] bf16, xa_bf [n,da] bf16, xbpb [n,db] f32,
            sa_wq/sa_wk/sa_wv/sa_wo [db,db] bf16, ca_wq [da,db] bf16,
            ca_wk/ca_wv/ca_wo [db,db] bf16, bq_sa [P,db/P] f32, bq_ca [P,db/P] f32
      outs: out [n,db] f32
    """
    nc = tc.nc
    KB, KA, NI = db // P, da // P, n // P
    MC = min(1024, n)         # projection m-chunk (columns of x^T); 2 psum banks
    NMC = n // MC
    PC = min(512, MC)         # one psum bank within a projection chunk
    NPC = MC // PC
    JH = min(1024, n)         # scores psum span (2 banks)
    NJH = n // JH
    JC = min(512, JH)         # one psum bank
    NJC = JH // JC
    SB = min(512, n)          # attention superblock (i columns per AV batch)
    NSB = n // SB
    IPSB = SB // P            # i-blocks per superblock
    EC = min(512, db)         # out-proj free chunk
    NEC = db // EC

    sc_sa = 1.0 / math.sqrt(float(db))
    sc_ca = 1.0 / math.sqrt(float(da))

    ctx = ExitStack()
    with ctx:
        sp = ctx.enter_context(tc.tile_pool(name="sp", bufs=1))
        pp = ctx.enter_context(tc.tile_pool(name="pp", bufs=1, space="PSUM"))
        dp = ctx.enter_context(tc.tile_pool(name="dp", bufs=1, space="DRAM"))

        # DRAM scratch
        qt_sa_d = dp.tile([db, n], BF16, tag="qt_sa")
        qt_ca_d = dp.tile([db, n], BF16, tag="qt_ca")
        xb1_d = dp.tile([n, db], F32, tag="xb1")
        xb1b_d = dp.tile([n, db], BF16, tag="xb1b")

        # persistent SBUF
        kT = sp.tile([P, KB, n], BF16, tag="kT")        # k^T  [feat, seq]
        v_sb = sp.tile([P, NI, db], BF16, tag="v")      # v    [seq, feat]
        bqs = sp.tile([P, KB], F32, tag="bqs")
        bqc = sp.tile([P, KB], F32, tag="bqc")
        zb = sp.tile([P, 1], F32, tag="zb")
        nc.sync.dma_start(bqs[:], ins["bq_sa"][:])
        nc.sync.dma_start(bqc[:], ins["bq_ca"][:])
        nc.gpsimd.memset(zb[:], 0.0)

        def load_w(name, ktiles):
            wt = sp.tile([P, ktiles, db], BF16, tag="w", bufs=2)
            nc.sync.dma_start(wt[:], ins[name].rearrange("(t p) e -> p t e", p=P))
            return wt

        def xpose_chunk(src_dram, ktiles, mcc):
            # x [mc-chunk, k] -> x^T chunk [p, kt, m] with k = kt*P + p
            # (tag shared with the attention wT superblock tiles: the phases
            # are sequential, and sharing keeps total SBUF under the cap)
            xT = sp.tile([P, ktiles, MC], BF16, tag="xcwt", bufs=2)
            nc.sync.dma_start_transpose(xT[:], src_dram[mcc * MC:(mcc + 1) * MC, :])
            return xT

        def proj_v(w_sb, src_dram, ktiles):
            # v[m, e] = sum_k x[m, k] w[k, e]  (natural layout, into v_sb).
            # One [P, db] psum spans all e-chunks: each LDWEIGHTS (the x-slice)
            # serves NEC matmuls instead of one.
            for mcc in range(NMC):
                xT = xpose_chunk(src_dram, ktiles, mcc)
                for q2 in range(MC // P):
                    mt = mcc * (MC // P) + q2
                    ps = pp.tile([P, db], F32, tag="ps_s", bufs=2)
                    for kt in range(ktiles):
                        for ecc in range(NEC):
                            nc.tensor.matmul(
                                ps[:, ecc * EC:(ecc + 1) * EC],
                                xT[:, kt, q2 * P:(q2 + 1) * P],
                                w_sb[:, kt, ecc * EC:(ecc + 1) * EC],
                                start=(kt == 0), stop=(kt == ktiles - 1),
                            )
                    nc.vector.tensor_copy(v_sb[:, mt, :], ps[:])

        def proj_T_block(w_sb, ktiles, xT, nt, mcc, sink):
            # out^T[f, m] = sum_k w[k, f] x^T[k, m] for f-tile nt, m-chunk mcc.
            # One [P, MC] psum spans NPC m-halves: each LDWEIGHTS (the w-slice)
            # serves NPC matmuls instead of one.
            ps = pp.tile([P, MC], F32, tag="ps_s", bufs=2)
            for kt in range(ktiles):
                for jc in range(NPC):
                    nc.tensor.matmul(
                        ps[:, jc * PC:(jc + 1) * PC],
                        w_sb[:, kt, nt * P:(nt + 1) * P],
                        xT[:, kt, jc * PC:(jc + 1) * PC],
                        start=(kt == 0), stop=(kt == ktiles - 1),
                    )
            sink(nt, mcc, ps)

        def q_sink(qt_d, bq_tile):
            def sink(nt, mcc, ps):
                qo = sp.tile([P, MC], BF16, tag="qv", bufs=2)
                nc.scalar.activation(qo[:], ps[:], AF.Identity, bias=bq_tile[:, nt:nt + 1])
                nc.sync.dma_start(qt_d[nt * P:(nt + 1) * P, mcc * MC:(mcc + 1) * MC], qo[:])
            return sink

        def k_sink(nt, mcc, ps):
            nc.vector.tensor_copy(kT[:, nt, mcc * MC:(mcc + 1) * MC], ps[:])

        def attention(qt_d, scale, wo_sb, resid_dram, writer):
            # Software-pipelined over superblocks: the scores/softmax/transpose
            # chain of superblock sbi is emitted BEFORE the AV/out-proj of
            # sbi-1, so the PE never stalls on the (ACT/DVE/DMA) softmax tail.
            def scores_phase(sbi, wt_t):
                for q3 in range(IPSB):
                    ib = sbi * IPSB + q3
                    qs_t = sp.tile([P, KB, P], BF16, tag="qs", bufs=2)
                    nc.sync.dma_start(
                        qs_t[:],
                        qt_d.rearrange("(t p) m -> p t m", p=P)[:, :, ib * P:(ib + 1) * P],
                    )
                    wb_t = sp.tile([P, n], BF16, tag="wb", bufs=2)
                    ss_t = sp.tile([P, NJH], F32, tag="ss", bufs=2)
                    for jh in range(NJH):
                        ps_s = pp.tile([P, JH], F32, tag="ps_s", bufs=2)
                        for kt in range(KB):
                            for jc in range(NJC):
                                nc.tensor.matmul(
                                    ps_s[:, jc * JC:(jc + 1) * JC],
                                    qs_t[:, kt, :],
                                    kT[:, kt, jh * JH + jc * JC:jh * JH + (jc + 1) * JC],
                                    start=(kt == 0), stop=(kt == KB - 1),
                                )
                        nc.scalar.activation(
                            wb_t[:, jh * JH:(jh + 1) * JH], ps_s[:], AF.Exp,
                            bias=zb[:], scale=scale,
                            accum_out=ss_t[:, jh:jh + 1],
                        )
                    rr = sp.tile([P, 1], F32, tag="rr", bufs=2)
                    if NJH > 1:
                        rs = sp.tile([P, 1], F32, tag="rs", bufs=2)
                        nc.vector.tensor_reduce(rs[:], ss_t[:], axis=mybir.AxisListType.X, op=ALU.add)
                        nc.vector.reciprocal(rr[:], rs[:])
                    else:
                        nc.vector.reciprocal(rr[:], ss_t[:])
                    nc.vector.tensor_scalar_mul(wb_t[:], wb_t[:], rr[:, 0:1])
                    # transpose the normalized weights: w[i, j] -> wT[j, i]
                    wtb = sp.tile([P, NI, P], BF16, tag="wtb", bufs=2)
                    nc.sync.dma_start_transpose(wtb[:], wb_t[:])
                    nc.vector.tensor_copy(wt_t[:, :, q3 * P:(q3 + 1) * P], wtb[:])

            def av_part(sbi, wt_t):
                # attn^T[d, i] = sum_j v[j, d] wT[j, i]
                at_t = sp.tile([P, KB, SB], BF16, tag="at", bufs=2)
                for dt in range(KB):
                    ps_a = pp.tile([P, SB], F32, tag="ps_a", bufs=2)
                    for jt in range(NI):
                        nc.tensor.matmul(
                            ps_a[:],
                            v_sb[:, jt, dt * P:(dt + 1) * P],
                            wt_t[:, jt, :],
                            start=(jt == 0), stop=(jt == NI - 1),
                        )
                    nc.vector.tensor_copy(at_t[:, dt, :], ps_a[:])
                return at_t

            def op_part(sbi, at_t):
                # out-proj + residual
                for q3 in range(IPSB):
                    ib = sbi * IPSB + q3
                    rx = sp.tile([P, db], F32, tag="rx", bufs=2)
                    nc.sync.dma_start(rx[:], resid_dram[ib * P:(ib + 1) * P, :])
                    ro = sp.tile([P, db], F32, tag="ro", bufs=2)
                    for ecc in range(NEC):
                        ps_o = pp.tile([P, EC], F32, tag="pj", bufs=2)
                        for dt in range(KB):
                            nc.tensor.matmul(
                                ps_o[:],
                                at_t[:, dt, q3 * P:(q3 + 1) * P],
                                wo_sb[:, dt, ecc * EC:(ecc + 1) * EC],
                                start=(dt == 0), stop=(dt == KB - 1),
                            )
                        nc.vector.tensor_tensor(
                            ro[:, ecc * EC:(ecc + 1) * EC], ps_o[:],
                            rx[:, ecc * EC:(ecc + 1) * EC], ALU.add,
                        )
                    writer(ib, ro)

            pend_av = None   # (sbi, wt_t) awaiting AV
            pend_op = None   # (sbi, at_t) awaiting out-proj
            for sbi in range(NSB):
                wt_t = sp.tile([P, NI, SB], BF16, tag="xcwt", bufs=2)
                scores_phase(sbi, wt_t)
                new_at = av_part(*pend_av) if pend_av is not None else None
                if pend_op is not None:
                    op_part(*pend_op)
                if new_at is not None:
                    pend_op = (pend_av[0], new_at)
                pend_av = (sbi, wt_t)
            at_t = av_part(*pend_av)
            if pend_op is not None:
                op_part(*pend_op)
            op_part(pend_av[0], at_t)

        def sa_writer(ib, ro):
            nc.sync.dma_start(xb1_d[ib * P:(ib + 1) * P, :], ro[:])
            rb = sp.tile([P, db], BF16, tag="rb", bufs=2)
            nc.scalar.activation(rb[:], ro[:], AF.Copy)
            nc.sync.dma_start(xb1b_d[ib * P:(ib + 1) * P, :], rb[:])

        def ca_writer(ib, ro):
            nc.sync.dma_start(outs["out"][ib * P:(ib + 1) * P, :], ro[:])

        # ===================== self-attention =====================
        wv = load_w("sa_wv", KB)
        proj_v(wv, ins["xb_bf"], KB)
        wq = load_w("sa_wq", KB)
        wk = load_w("sa_wk", KB)
        sink_q_sa = q_sink(qt_sa_d, bqs)
        for mcc in range(NMC):
            xT = xpose_chunk(ins["xb_bf"], KB, mcc)
            for nt in range(KB):
                proj_T_block(wq, KB, xT, nt, mcc, sink_q_sa)
                proj_T_block(wk, KB, xT, nt, mcc, k_sink)
        wo = load_w("sa_wo", KB)
        attention(qt_sa_d, sc_sa, wo, ins["xbpb"], sa_writer)

        # ===================== cross-attention =====================
        wv2 = load_w("ca_wv", KB)
        proj_v(wv2, xb1b_d, KB)
        wk2 = load_w("ca_wk", KB)
        sink_q_ca = q_sink(qt_ca_d, bqc)
        for mcc in range(NMC):
            xTb = xpose_chunk(xb1b_d, KB, mcc)
            for nt in range(KB):
                proj_T_block(wk2, KB, xTb, nt, mcc, k_sink)
        wq2 = load_w("ca_wq", KA)
        for mcc in range(NMC):
            xTa = xpose_chunk(ins["xa_bf"], KA, mcc)
            for nt in range(KB):
                proj_T_block(wq2, KA, xTa, nt, mcc, sink_q_ca)
        wo2 = load_w("ca_wo", KB)
        attention(qt_ca_d, sc_ca, wo2, xb1_d, ca_writer)


def build_program(n=N_FULL, da=DA_FULL, db=DB_FULL, repeat=1):
    """Build the single-core Bass program; returns the Bass module.

    repeat>1 re-emits the whole block body N times (idempotent — same inputs
    and scratch): used to measure per-iteration device time above the fixed
    dispatch overhead."""
    nc = bacc.Bacc("TRN2", target_bir_lowering=False, debug=False, enable_asserts=False)
    KB = db // P
    ins = {
        "xb_bf": nc.dram_tensor("xb_bf", [n, db], BF16, kind="ExternalInput").ap(),
        "xa_bf": nc.dram_tensor("xa_bf", [n, da], BF16, kind="ExternalInput").ap(),
        "xbpb": nc.dram_tensor("xbpb", [n, db], F32, kind="ExternalInput").ap(),
        "sa_wq": nc.dram_tensor("sa_wq", [db, db], BF16, kind="ExternalInput").ap(),
        "sa_wk": nc.dram_tensor("sa_wk", [db, db], BF16, kind="ExternalInput").ap(),
        "sa_wv": nc.dram_tensor("sa_wv", [db, db], BF16, kind="ExternalInput").ap(),
        "sa_wo": nc.dram_tensor("sa_wo", [db, db], BF16, kind="ExternalInput").ap(),
        "ca_wq": nc.dram_tensor("ca_wq", [da, db], BF16, kind="ExternalInput").ap(),
        "ca_wk": nc.dram_tensor("ca_wk", [db, db], BF16, kind="ExternalInput").ap(),
        "ca_wv": nc.dram_tensor("ca_wv", [db, db], BF16, kind="ExternalInput").ap(),
        "ca_wo": nc.dram_tensor("ca_wo", [db, db], BF16, kind="ExternalInput").ap(),
        "bq_sa": nc.dram_tensor("bq_sa", [P, KB], F32, kind="ExternalInput").ap(),
        "bq_ca": nc.dram_tensor("bq_ca", [P, KB], F32, kind="ExternalInput").ap(),
    }
    outs = {"out": nc.dram_tensor("out", [n, db], F32, kind="ExternalOutput").ap()}
    with tile.TileContext(nc) as tc:
        for _ in range(repeat):
            build_block(tc, outs, ins, n, da, db)
    nc.compile()
    return nc


def prepare_maps(inputs, n=N_FULL, da=DA_FULL, db=DB_FULL):
    """Host-side prep: bf16 casts + exact bias folding.  Returns (in_maps, add_out)."""
    bf = ml_dtypes.bfloat16
    f32 = np.float32
    g = {k: np.ascontiguousarray(np.asarray(v)) for k, v in inputs.items()}
    nb = g["x_a"].shape[0]

    # exact folds (see module docstring); all biases are added in fp32
    b_eff_sa = (g["sa_bv"].astype(f32) @ g["sa_wo"].astype(f32) + g["sa_bo"].astype(f32))
    b_eff_ca = (g["ca_bv"].astype(f32) @ g["ca_wo"].astype(f32) + g["ca_bo"].astype(f32))
    xbpb = (g["x_b"].astype(f32) + b_eff_sa[None, None, :]).astype(f32)

    KB = db // P
    common = {
        "sa_wq": g["sa_wq"].astype(bf), "sa_wk": g["sa_wk"].astype(bf),
        "sa_wv": g["sa_wv"].astype(bf), "sa_wo": g["sa_wo"].astype(bf),
        "ca_wq": g["ca_wq"].astype(bf), "ca_wk": g["ca_wk"].astype(bf),
        "ca_wv": g["ca_wv"].astype(bf), "ca_wo": g["ca_wo"].astype(bf),
        "bq_sa": np.ascontiguousarray(g["sa_bq"].astype(f32).reshape(KB, P).T),
        "bq_ca": np.ascontiguousarray(g["ca_bq"].astype(f32).reshape(KB, P).T),
    }
    in_maps = []
    for b in range(nb):
        in_maps.append(dict(
            xb_bf=g["x_b"][b].astype(bf),
            xa_bf=g["x_a"][b].astype(bf),
            xbpb=np.ascontiguousarray(xbpb[b]),
            **common,
        ))
    return in_maps, b_eff_ca


_CACHE = {}


def run_on_device(inputs, trace=False, **run_kwargs):
    """Run the full problem on 8 NeuronCores.  Returns (out [B,N,DB] f32, results)."""
    if not trace:
        # NTFF tracing needs antenv.axon_hooks, absent in this container; make
        # sure an inherited BASS_TRACE=1 can't route us into that path.
        os.environ.setdefault("BASS_NEVER_TRACE", "1")
    if "nc" not in _CACHE:
        _CACHE["nc"] = build_program()
    nc = _CACHE["nc"]
    in_maps, add_out = prepare_maps(inputs)
    res = run_bass_kernel_spmd(
        nc, in_maps, core_ids=list(range(len(in_maps))), trace=trace, **run_kwargs,
    )
    out = np.stack([r["out"] for r in res.results], axis=0)
    out = (out + add_out[None, None, :]).astype(np.float32)
    return out, res


def kernel(**inputs) -> np.ndarray:
    out, _ = run_on_device(inputs)
    return out


# revision 23
# speedup vs baseline: 78.7902x; 1.0556x over previous
"""Trainium2 Bass kernel: dual-attention transformer block (nn_CustomBlock).

Reference semantics (per batch element b):
    q/k/v = x_b @ sa_w{q,k,v} + sa_b{q,k,v}
    sa    = softmax(q k^T / sqrt(DB)) v @ sa_wo + sa_bo
    x_b1  = x_b + sa
    q     = x_a @ ca_wq + ca_bq ; k/v = x_b1 @ ca_w{k,v} + ca_b{k,v}
    out   = x_b1 + softmax(q k^T / sqrt(DA)) v @ ca_wo + ca_bo

Sharding: data-parallel over batch — 8 batch elements, one per NeuronCore,
weights replicated.  No collectives.

Device kernel works in bf16 for all matmul operands (fp32 PSUM accumulation,
fp32 residual stream).  Exact host-side bias folding:
  - k-bias shifts every score row by a constant -> softmax-invariant -> dropped.
  - v-bias passes through attention unchanged (softmax weights sum to 1), so
    bv @ wo + bo folds into a single per-feature vector added to the residual
    input (SA) / the final output (CA) on the host.
  - q-bias is applied on device (per-partition bias in the q^T layout).

Softmax skips the max-subtraction: scores = q.k/sqrt(D) with these operand
scales stays in [-3, 3]; exp() in fp32 is safe by a wide margin.
"""

import math
import os
from contextlib import ExitStack

import numpy as np
import ml_dtypes

import concourse.bass as bass
import concourse.mybir as mybir
import concourse.tile as tile
from concourse import bacc
from concourse.bass_utils import run_bass_kernel_spmd

P = 128
F32 = mybir.dt.float32
BF16 = mybir.dt.bfloat16
AF = mybir.ActivationFunctionType
ALU = mybir.AluOpType

B_FULL, N_FULL, DA_FULL, DB_FULL = 8, 2048, 768, 1024


def build_block(tc, outs, ins, n, da, db):
    """Emit the dual-attention block into TileContext `tc`.

    ins/outs: dicts of DRAM APs:
      ins:  xb_bf [n,db] bf16, xa_bf [n,da] bf16, xbpb [n,db] f32,
            sa_wq/sa_wk/sa_wv/sa_wo [db,db] bf16, ca_wq [da,db] bf16,
            ca_wk/ca_wv/ca_wo [db,db] bf16, bq_sa [P,db/P] f32, bq_ca [P,db/P] f32
      outs: out [n,db] f32
    """
    nc = tc.nc
    KB, KA, NI = db // P, da // P, n // P
    MC = min(1024, n)         # projection m-chunk (columns of x^T); 2 psum banks
    NMC = n // MC
    PC = min(512, MC)         # one psum bank within a projection chunk
    NPC = MC // PC
    JH = min(1024, n)         # scores psum span (2 banks)
    NJH = n // JH
    JC = min(512, JH)         # one psum bank
    NJC = JH // JC
    SB = min(512, n)          # attention superblock (i columns per AV batch)
    NSB = n // SB
    IPSB = SB // P            # i-blocks per superblock
    EC = min(512, db)         # out-proj free chunk
    NEC = db // EC

    sc_sa = 1.0 / math.sqrt(float(db))
    sc_ca = 1.0 / math.sqrt(float(da))

    ctx = ExitStack()
    with ctx:
        sp = ctx.enter_context(tc.tile_pool(name="sp", bufs=1))
        pp = ctx.enter_context(tc.tile_pool(name="pp", bufs=1, space="PSUM"))
        dp = ctx.enter_context(tc.tile_pool(name="dp", bufs=1, space="DRAM"))

        # DRAM scratch
        qt_sa_d = dp.tile([db, n], BF16, tag="qt_sa")
        qt_ca_d = dp.tile([db, n], BF16, tag="qt_ca")
        xb1_d = dp.tile([n, db], F32, tag="xb1")
        xb1b_d = dp.tile([n, db], BF16, tag="xb1b")

        # persistent SBUF
        kT = sp.tile([P, KB, n], BF16, tag="kT")        # k^T  [feat, seq]
        v_sb = sp.tile([P, NI, db], BF16, tag="v")      # v    [seq, feat]
        bqs = sp.tile([P, KB], F32, tag="bqs")
        bqc = sp.tile([P, KB], F32, tag="bqc")
        zb = sp.tile([P, 1], F32, tag="zb")
        nc.sync.dma_start(bqs[:], ins["bq_sa"][:])
        nc.sync.dma_start(bqc[:], ins["bq_ca"][:])
        nc.gpsimd.memset(zb[:], 0.0)

        def load_w(name, ktiles):
            wt = sp.tile([P, ktiles, db], BF16, tag="w", bufs=2)
            nc.sync.dma_start(wt[:], ins[name].rearrange("(t p) e -> p t e", p=P))
            return wt

        def xpose_chunk(src_dram, ktiles, mcc):
            # x [mc-chunk, k] -> x^T chunk [p, kt, m] with k = kt*P + p
            # (tag shared with the attention wT superblock tiles: the phases
            # are sequential, and sharing keeps total SBUF under the cap)
            xT = sp.tile([P, ktiles, MC], BF16, tag="xcwt", bufs=2)
            nc.sync.dma_start_transpose(xT[:], src_dram[mcc * MC:(mcc + 1) * MC, :])
            return xT

        def proj_v(w_sb, src_dram, ktiles):
            # v[m, e] = sum_k x[m, k] w[k, e]  (natural layout, into v_sb).
            # One [P, db] psum spans all e-chunks: each LDWEIGHTS (the x-slice)
            # serves NEC matmuls instead of one.
            for mcc in range(NMC):
                xT = xpose_chunk(src_dram, ktiles, mcc)
                for q2 in range(MC // P):
                    mt = mcc * (MC // P) + q2
                    ps = pp.tile([P, db], F32, tag="ps_s", bufs=2)
                    for kt in range(ktiles):
                        for ecc in range(NEC):
                            nc.tensor.matmul(
                                ps[:, ecc * EC:(ecc + 1) * EC],
                                xT[:, kt, q2 * P:(q2 + 1) * P],
                                w_sb[:, kt, ecc * EC:(ecc + 1) * EC],
                                start=(kt == 0), stop=(kt == ktiles - 1),
                            )
                    nc.vector.tensor_copy(v_sb[:, mt, :], ps[:])

        def proj_T_block(w_sb, ktiles, xT, nt, mcc, sink):
            # out^T[f, m] = sum_k w[k, f] x^T[k, m] for f-tile nt, m-chunk mcc.
            # One [P, MC] psum spans NPC m-halves: each LDWEIGHTS (the w-slice)
            # serves NPC matmuls instead of one.
            ps = pp.tile([P, MC], F32, tag="ps_s", bufs=2)
            for kt in range(ktiles):
                for jc in range(NPC):
                    nc.tensor.matmul(
                        ps[:, jc * PC:(jc + 1) * PC],
                        w_sb[:, kt, nt * P:(nt + 1) * P],
                        xT[:, kt, jc * PC:(jc + 1) * PC],
                        start=(kt == 0), stop=(kt == ktiles - 1),
                    )
            sink(nt, mcc, ps)

        def q_sink(qt_d, bq_tile):
            def sink(nt, mcc, ps):
                qo = sp.tile([P, MC], BF16, tag="qv", bufs=2)
                nc.scalar.activation(qo[:], ps[:], AF.Identity, bias=bq_tile[:, nt:nt + 1])
                nc.sync.dma_start(qt_d[nt * P:(nt + 1) * P, mcc * MC:(mcc + 1) * MC], qo[:])
            return sink

        def k_sink(nt, mcc, ps):
            nc.vector.tensor_copy(kT[:, nt, mcc * MC:(mcc + 1) * MC], ps[:])

        def attention(qt_d, scale, wo_sb, resid_dram, writer):
            # Software-pipelined over superblocks: the scores/softmax/transpose
            # chain of superblock sbi is emitted BEFORE the AV/out-proj of
            # sbi-1, so the PE never stalls on the (ACT/DVE/DMA) softmax tail.
            def scores_phase(sbi, wt_t):
                for q3 in range(IPSB):
                    ib = sbi * IPSB + q3
                    qs_t = sp.tile([P, KB, P], BF16, tag="qs", bufs=2)
                    nc.sync.dma_start(
                        qs_t[:],
                        qt_d.rearrange("(t p) m -> p t m", p=P)[:, :, ib * P:(ib + 1) * P],
                    )
                    wb_t = sp.tile([P, n], BF16, tag="wb", bufs=2)
                    ss_t = sp.tile([P, NJH], F32, tag="ss", bufs=2)
                    for jh in range(NJH):
                        ps_s = pp.tile([P, JH], F32, tag="ps_s", bufs=2)
                        for kt in range(KB):
                            for jc in range(NJC):
                                nc.tensor.matmul(
                                    ps_s[:, jc * JC:(jc + 1) * JC],
                                    qs_t[:, kt, :],
                                    kT[:, kt, jh * JH + jc * JC:jh * JH + (jc + 1) * JC],
                                    start=(kt == 0), stop=(kt == KB - 1),
                                )
                        nc.scalar.activation(
                            wb_t[:, jh * JH:(jh + 1) * JH], ps_s[:], AF.Exp,
                            bias=zb[:], scale=scale,
                            accum_out=ss_t[:, jh:jh + 1],
                        )
                    rr = sp.tile([P, 1], F32, tag="rr", bufs=2)
                    if NJH > 1:
                        rs = sp.tile([P, 1], F32, tag="rs", bufs=2)
                        nc.vector.tensor_reduce(rs[:], ss_t[:], axis=mybir.AxisListType.X, op=ALU.add)
                        nc.vector.reciprocal(rr[:], rs[:])
                    else:
                        nc.vector.reciprocal(rr[:], ss_t[:])
                    nc.vector.tensor_scalar_mul(wb_t[:], wb_t[:], rr[:, 0:1])
                    # transpose the normalized weights: w[i, j] -> wT[j, i]
                    wtb = sp.tile([P, NI, P], BF16, tag="wtb", bufs=2)
                    nc.sync.dma_start_transpose(wtb[:], wb_t[:])
                    nc.vector.tensor_copy(wt_t[:, :, q3 * P:(q3 + 1) * P], wtb[:])

            def av_part(sbi, wt_t):
                # attn^T[d, i] = sum_j v[j, d] wT[j, i]
                at_t = sp.tile([P, KB, SB], BF16, tag="at", bufs=2)
                for dt in range(KB):
                    ps_a = pp.tile([P, SB], F32, tag="ps_a", bufs=2)
                    for jt in range(NI):
                        nc.tensor.matmul(
                            ps_a[:],
                            v_sb[:, jt, dt * P:(dt + 1) * P],
                            wt_t[:, jt, :],
                            start=(jt == 0), stop=(jt == NI - 1),
                        )
                    nc.vector.tensor_copy(at_t[:, dt, :], ps_a[:])
                return at_t

            def op_part(sbi, at_t):
                # out-proj + residual
                for q3 in range(IPSB):
                    ib = sbi * IPSB + q3
                    rx = sp.tile([P, db], F32, tag="rx", bufs=2)
                    nc.sync.dma_start(rx[:], resid_dram[ib * P:(ib + 1) * P, :])
                    ro = sp.tile([P, db], F32, tag="ro", bufs=2)
                    for ecc in range(NEC):
                        ps_o = pp.tile([P, EC], F32, tag="pj", bufs=2)
                        for dt in range(KB):
                            nc.tensor.matmul(
                                ps_o[:],
                                at_t[:, dt, q3 * P:(q3 + 1) * P],
                                wo_sb[:, dt, ecc * EC:(ecc + 1) * EC],
                                start=(dt == 0), stop=(dt == KB - 1),
                            )
                        nc.vector.tensor_tensor(
                            ro[:, ecc * EC:(ecc + 1) * EC], ps_o[:],
                            rx[:, ecc * EC:(ecc + 1) * EC], ALU.add,
                        )
                    writer(ib, ro)

            pend_av = None   # (sbi, wt_t) awaiting AV
            pend_op = None   # (sbi, at_t) awaiting out-proj
            for sbi in range(NSB):
                wt_t = sp.tile([P, NI, SB], BF16, tag="xcwt", bufs=2)
                scores_phase(sbi, wt_t)
                new_at = av_part(*pend_av) if pend_av is not None else None
                if pend_op is not None:
                    op_part(*pend_op)
                if new_at is not None:
                    pend_op = (pend_av[0], new_at)
                pend_av = (sbi, wt_t)
            at_t = av_part(*pend_av)
            if pend_op is not None:
                op_part(*pend_op)
            op_part(pend_av[0], at_t)

        def sa_writer(ib, ro):
            nc.sync.dma_start(xb1_d[ib * P:(ib + 1) * P, :], ro[:])
            rb = sp.tile([P, db], BF16, tag="rb", bufs=2)
            nc.scalar.activation(rb[:], ro[:], AF.Copy)
            nc.sync.dma_start(xb1b_d[ib * P:(ib + 1) * P, :], rb[:])

        def ca_writer(ib, ro):
            nc.sync.dma_start(outs["out"][ib * P:(ib + 1) * P, :], ro[:])

        # ===================== self-attention =====================
        wv = load_w("sa_wv", KB)
        proj_v(wv, ins["xb_bf"], KB)
        wq = load_w("sa_wq", KB)
        wk = load_w("sa_wk", KB)
        sink_q_sa = q_sink(qt_sa_d, bqs)
        for mcc in range(NMC):
            xT = xpose_chunk(ins["xb_bf"], KB, mcc)
            for nt in range(KB):
                proj_T_block(wq, KB, xT, nt, mcc, sink_q_sa)
                proj_T_block(wk, KB, xT, nt, mcc, k_sink)
        # CA-q depends only on x_a — emit it before SA attention, where the
        # chunk ring is free; it decouples the SA->CA boundary entirely.
        wq2 = load_w("ca_wq", KA)
        sink_q_ca = q_sink(qt_ca_d, bqc)
        for mcc in range(NMC):
            xTa = xpose_chunk(ins["xa_bf"], KA, mcc)
            for nt in range(KB):
                proj_T_block(wq2, KA, xTa, nt, mcc, sink_q_ca)

        wo = load_w("sa_wo", KB)
        attention(qt_sa_d, sc_sa, wo, ins["xbpb"], sa_writer)

        # ===================== cross-attention =====================
        # v and k share each transposed xb1 chunk (one transpose instead of
        # two, and 2x the PE work per chunk keeps the chunk ring ahead).
        wv2 = load_w("ca_wv", KB)
        wk2 = load_w("ca_wk", KB)
        for mcc in range(NMC):
            xTb = xpose_chunk(xb1b_d, KB, mcc)
            for q2 in range(MC // P):
                mt = mcc * (MC // P) + q2
                ps = pp.tile([P, db], F32, tag="ps_s", bufs=2)
                for kt in range(KB):
                    for ecc in range(NEC):
                        nc.tensor.matmul(
                            ps[:, ecc * EC:(ecc + 1) * EC],
                            xTb[:, kt, q2 * P:(q2 + 1) * P],
                            wv2[:, kt, ecc * EC:(ecc + 1) * EC],
                            start=(kt == 0), stop=(kt == KB - 1),
                        )
                nc.vector.tensor_copy(v_sb[:, mt, :], ps[:])
            for nt in range(KB):
                proj_T_block(wk2, KB, xTb, nt, mcc, k_sink)
        wo2 = load_w("ca_wo", KB)
        attention(qt_ca_d, sc_ca, wo2, xb1_d, ca_writer)


def build_program(n=N_FULL, da=DA_FULL, db=DB_FULL, repeat=1):
    """Build the single-core Bass program; returns the Bass module.

    repeat>1 re-emits the whole block body N times (idempotent — same inputs
    and scratch): used to measure per-iteration device time above the fixed
    dispatch overhead."""
    nc = bacc.Bacc("TRN2", target_bir_lowering=False, debug=False, enable_asserts=False)
    KB = db // P
    ins = {
        "xb_bf": nc.dram_tensor("xb_bf", [n, db], BF16, kind="ExternalInput").ap(),
        "xa_bf": nc.dram_tensor("xa_bf", [n, da], BF16, kind="ExternalInput").ap(),
        "xbpb": nc.dram_tensor("xbpb", [n, db], F32, kind="ExternalInput").ap(),
        "sa_wq": nc.dram_tensor("sa_wq", [db, db], BF16, kind="ExternalInput").ap(),
        "sa_wk": nc.dram_tensor("sa_wk", [db, db], BF16, kind="ExternalInput").ap(),
        "sa_wv": nc.dram_tensor("sa_wv", [db, db], BF16, kind="ExternalInput").ap(),
        "sa_wo": nc.dram_tensor("sa_wo", [db, db], BF16, kind="ExternalInput").ap(),
        "ca_wq": nc.dram_tensor("ca_wq", [da, db], BF16, kind="ExternalInput").ap(),
        "ca_wk": nc.dram_tensor("ca_wk", [db, db], BF16, kind="ExternalInput").ap(),
        "ca_wv": nc.dram_tensor("ca_wv", [db, db], BF16, kind="ExternalInput").ap(),
        "ca_wo": nc.dram_tensor("ca_wo", [db, db], BF16, kind="ExternalInput").ap(),
        "bq_sa": nc.dram_tensor("bq_sa", [P, KB], F32, kind="ExternalInput").ap(),
        "bq_ca": nc.dram_tensor("bq_ca", [P, KB], F32, kind="ExternalInput").ap(),
    }
    outs = {"out": nc.dram_tensor("out", [n, db], F32, kind="ExternalOutput").ap()}
    with tile.TileContext(nc) as tc:
        for _ in range(repeat):
            build_block(tc, outs, ins, n, da, db)
    nc.compile()
    return nc


def prepare_maps(inputs, n=N_FULL, da=DA_FULL, db=DB_FULL):
    """Host-side prep: bf16 casts + exact bias folding.  Returns (in_maps, add_out)."""
    bf = ml_dtypes.bfloat16
    f32 = np.float32
    g = {k: np.ascontiguousarray(np.asarray(v)) for k, v in inputs.items()}
    nb = g["x_a"].shape[0]

    # exact folds (see module docstring); all biases are added in fp32
    b_eff_sa = (g["sa_bv"].astype(f32) @ g["sa_wo"].astype(f32) + g["sa_bo"].astype(f32))
    b_eff_ca = (g["ca_bv"].astype(f32) @ g["ca_wo"].astype(f32) + g["ca_bo"].astype(f32))
    xbpb = (g["x_b"].astype(f32) + b_eff_sa[None, None, :]).astype(f32)

    KB = db // P
    common = {
        "sa_wq": g["sa_wq"].astype(bf), "sa_wk": g["sa_wk"].astype(bf),
        "sa_wv": g["sa_wv"].astype(bf), "sa_wo": g["sa_wo"].astype(bf),
        "ca_wq": g["ca_wq"].astype(bf), "ca_wk": g["ca_wk"].astype(bf),
        "ca_wv": g["ca_wv"].astype(bf), "ca_wo": g["ca_wo"].astype(bf),
        "bq_sa": np.ascontiguousarray(g["sa_bq"].astype(f32).reshape(KB, P).T),
        "bq_ca": np.ascontiguousarray(g["ca_bq"].astype(f32).reshape(KB, P).T),
    }
    in_maps = []
    for b in range(nb):
        in_maps.append(dict(
            xb_bf=g["x_b"][b].astype(bf),
            xa_bf=g["x_a"][b].astype(bf),
            xbpb=np.ascontiguousarray(xbpb[b]),
            **common,
        ))
    return in_maps, b_eff_ca


_CACHE = {}


def run_on_device(inputs, trace=False, **run_kwargs):
    """Run the full problem on 8 NeuronCores.  Returns (out [B,N,DB] f32, results)."""
    if not trace:
        # NTFF tracing needs antenv.axon_hooks, absent in this container; make
        # sure an inherited BASS_TRACE=1 can't route us into that path.
        os.environ.setdefault("BASS_NEVER_TRACE", "1")
    if "nc" not in _CACHE:
        _CACHE["nc"] = build_program()
    nc = _CACHE["nc"]
    in_maps, add_out = prepare_maps(inputs)
    res = run_bass_kernel_spmd(
        nc, in_maps, core_ids=list(range(len(in_maps))), trace=trace, **run_kwargs,
    )
    out = np.stack([r["out"] for r in res.results], axis=0)
    out = (out + add_out[None, None, :]).astype(np.float32)
    return out, res


def kernel(**inputs) -> np.ndarray:
    out, _ = run_on_device(inputs)
    return out
